# revision 49
# baseline (speedup 1.0000x reference)
"""GCN (2-layer, PyG GCNConv-style) on 8 Trainium2 NeuronCores.

Strategy (1D destination partition, per sharding hint):
  - Nodes are relabeled into a "virtual" order: 8 NCs x 8 Q7-cores x NSLOT
    slots. Each (NC, core) owns ~1563 original nodes.
  - Edges are grouped by destination core ("edge lists grouped by
    destination-node partition") and sorted by destination within the core.
  - GCNConv is linear before the nonlinearity, so aggregation happens in the
    2-dim input space (layer 1: aggregate dinv*x, then @W1) and in the 1-dim
    output space (layer 2: aggregate dinv*(h1@W2)).
  - Per-edge gather of source values runs on GPSIMD ap_gather with sixteen
    per-partition sub-tables; a shipped 0/1 mask + one block-diagonal PE
    matmul select the correct sub-table and reduce 16 partitions -> 1 row.
    Pass B packs the 2-dim y values as bf16 pairs (one 4-byte gather unit
    per edge instead of two), halving the ucode's per-index inner loop.
  - Segment sums use chunked prefix scans (DVE) over per-core streams plus
    boundary gathers of the scan table; destination degrees come from
    boundary differences. Chunk size C is fitted to the largest per-core
    stream (minimal padding); gather outputs are double-buffered and the
    boundary-gather reloads alternate between two SBUF slots so DMA
    overlaps GPSIMD.
  - dinv and g are exchanged across the 8 cores with AllGather collectives.
Host code does only data movement: permutations, grouping, padding, index
tables, and broadcast of the tiny weights.
"""

import math

import numpy as np

N_CORES = 8
N = 100_000
IN_DIM = 2
HID = 64
C_TARGET = 3584

_cache = {}


def _ceil16(x):
    return ((x + 15) // 16) * 16


def _prep(x, edge_index, W1, b1, W2, b2):
    row = np.asarray(edge_index[0], dtype=np.int64)
    col = np.asarray(edge_index[1], dtype=np.int64)
    E = row.shape[0]

    # ---- node -> (nc, core, j) assignment ----
    per_nc = (N + N_CORES - 1) // N_CORES  # 12500
    nd_core = np.full(8, per_nc // 8, dtype=np.int64)
    nd_core[: per_nc % 8] += 1  # [1563]*4 + [1562]*4
    cum_nd = np.concatenate([[0], np.cumsum(nd_core)])  # [9]

    v = np.arange(N, dtype=np.int64)
    nc_of = v // per_nc
    l_of = v % per_nc
    core_of = np.searchsorted(cum_nd, l_of, side="right") - 1
    j_of = l_of - cum_nd[core_of]
    cg_of = nc_of * 8 + core_of  # global core id [0,64)

    # ---- edge stream: group by dest core, sort by dest j ----
    e_cg = cg_of[col]
    e_j = j_of[col]
    order = np.lexsort((e_j, e_cg))
    s_cg = e_cg[order]
    s_j = e_j[order]
    s_src = row[order]

    S_real = np.bincount(s_cg, minlength=64)
    cg_start = np.concatenate([[0], np.cumsum(S_real)])
    # tight chunking: same chunk count as C_TARGET would give, but C
    # shrunk to just cover the largest per-core stream (less padding)
    maxS = int(S_real.max())
    n_chunks = int(math.ceil(maxS / C_TARGET))
    C = _ceil16(int(math.ceil(maxS / n_chunks)))
    S_pad = n_chunks * C

    # ---- boundaries per core ----
    # counts per (cg, j); nd = nd_core[c]
    bounds = []  # per cg: array length nd+1
    for cg in range(64):
        c = cg % 8
        nd = int(nd_core[c])
        jj = s_j[cg_start[cg] : cg_start[cg + 1]]
        cnt = np.bincount(jj, minlength=nd)
        bounds.append(np.concatenate([[0], np.cumsum(cnt)]))

    # chunk assignment + B_cap
    maxb = 0
    for cg in range(64):
        b = bounds[cg]
        kb = np.minimum(b // C, n_chunks - 1)
        maxb = max(maxb, int(np.bincount(kb, minlength=n_chunks).max()))
    B_cap = _ceil16(maxb + 2)
    NB = n_chunks * B_cap
    NPP = (NB + 15) // 16
    NSLOT = 16 * NPP
    VN = 64 * NSLOT
    SUB = VN // 16
    assert SUB * 2 <= 32768, (SUB, NB)

    # ---- padded boundary lists (PBL), positions, virtual ids ----
    PBL = np.zeros((64, NB), dtype=np.int64)
    pos_of = np.zeros((64,), dtype=object)
    for cg in range(64):
        b = bounds[cg]
        kb = np.minimum(b // C, n_chunks - 1)
        cnts = np.bincount(kb, minlength=n_chunks)
        lists = []
        last_val = 0
        start = 0
        for k in range(n_chunks):
            ck = int(cnts[k])
            vals = b[start : start + ck]
            start += ck
            if ck > 0:
                last_val = int(vals[-1])
                padv = last_val
            else:
                padv = max(k * C, last_val)
            lst = np.concatenate([vals, np.full(B_cap - ck, padv, dtype=np.int64)])
            lists.append(lst)
        PBL[cg] = np.concatenate(lists)
        # entry position of b[j] in PBL: P[j] = j + padcum[kb[j]]
        pads = B_cap - cnts
        padcum = np.concatenate([[0], np.cumsum(pads)])[:-1]
        P = np.arange(len(b)) + padcum[kb]
        pos = P[1:] - 1  # pos_j for j = 0..nd-1
        assert pos.max() <= NB - 2, (cg, pos.max(), NB)
        pos_of[cg] = pos

    # virtual id per original node
    virt = np.zeros(N, dtype=np.int64)
    for cg in range(64):
        c = cg % 8
        nd = int(nd_core[c])
        sel = cg_of == cg
        virt[sel] = cg * NSLOT + pos_of[cg][j_of[sel]]

    # ---- per-edge source virtual ids, padded streams ----
    su = virt[s_src]
    su_stream = np.zeros((64, S_pad), dtype=np.int64)
    for cg in range(64):
        n = int(S_real[cg])
        su_stream[cg, :n] = su[cg_start[cg] : cg_start[cg + 1]]

    # ---- shipped arrays per NC ----
    x = np.asarray(x, dtype=np.float32)
    x_virt = np.zeros((VN, 2), dtype=np.float32)
    x_virt[virt] = x

    qv = (su_stream // SUB).astype(np.int64)  # [64, S_pad] in [0,16)
    idxv = (su_stream % SUB).astype(np.int16)

    import ml_dtypes

    # (hi - lo) per virtual slot for ALL 64 cores, in the [1024, NPP]
    # layout an AllGather of per-core [128, NPP] dinv shards would produce
    degm1_full = np.zeros((1024, NPP), dtype=np.float32)
    for i2 in range(N_CORES):
        for c2 in range(8):
            cg2 = i2 * 8 + c2
            pbl_e = np.concatenate([PBL[cg2], PBL[cg2][-1:]])
            dm1 = (pbl_e[1 : NSLOT + 1] - pbl_e[:NSLOT]).astype(np.float32)
            degm1_full[i2 * 128 + 16 * c2 : i2 * 128 + 16 * c2 + 16] = (
                dm1.reshape(16, NPP)
            )

    in_maps = []
    for i in range(N_CORES):
        idx16 = np.zeros((n_chunks, 128, C // 16), dtype=np.int16)
        maskf = np.zeros((n_chunks, 128, C), dtype=np.float32)
        bidx16 = np.zeros((n_chunks, 128, B_cap // 16), dtype=np.int16)
        lo = np.zeros((128, NPP), dtype=np.float32)
        hi = np.zeros((128, NPP), dtype=np.float32)
        x_own = np.zeros((128, 2 * NPP), dtype=np.float32)
        for c in range(8):
            cg = i * 8 + c
            for k in range(n_chunks):
                chunk_idx = idxv[cg, k * C : (k + 1) * C].reshape(C // 16, 16)
                idx16[k, 16 * c : 16 * c + 16, :] = chunk_idx.T
                qk = qv[cg, k * C : (k + 1) * C]
                # mask[16c+p, s] = (q[s] == p), 0 for dummy slots
                s_valid = (np.arange(k * C, (k + 1) * C) < S_real[cg]).astype(
                    np.float32
                )
                m = (qk[None, :] == np.arange(16)[:, None]).astype(np.float32)
                maskf[k, 16 * c : 16 * c + 16, :] = m * s_valid[None, :]
                pb = PBL[cg, k * B_cap : (k + 1) * B_cap] - k * C
                assert pb.min() >= 0 and pb.max() <= C, (cg, k)
                bidx16[k, 16 * c : 16 * c + 16, :] = (
                    pb.astype(np.int16).reshape(B_cap // 16, 16).T
                )
            pbl_ext = np.concatenate([PBL[cg], PBL[cg][-1:]])
            lo_full = pbl_ext[:NSLOT].astype(np.float32)
            hi_full = pbl_ext[1 : NSLOT + 1].astype(np.float32)
            lo[16 * c : 16 * c + 16] = lo_full.reshape(16, NPP)
            hi[16 * c : 16 * c + 16] = hi_full.reshape(16, NPP)
            x_own[16 * c : 16 * c + 16] = x_virt[
                cg * NSLOT : (cg + 1) * NSLOT
            ].reshape(16, 2 * NPP)
        # pass-B mask in bf16 with each slot duplicated for the packed
        # (y0, y1) bf16 pair layout of the gathered stream
        maskb = np.repeat(maskf, 2, axis=-1).astype(ml_dtypes.bfloat16)
        in_maps.append(
            {
                "idx16": idx16,
                "maskb": maskb,
                "maskf": maskf,
                "bidx16": bidx16,
                "pbl_lo": lo,
                "pbl_hi": hi,
                "x_own": x_own,
                "x_virt": x_virt,
                "degm1_full": degm1_full,
                "w1b0": np.broadcast_to(
                    np.asarray(W1, np.float32)[0], (128, HID)
                ).copy(),
                "w1b1": np.broadcast_to(
                    np.asarray(W1, np.float32)[1], (128, HID)
                ).copy(),
                "b1b": np.broadcast_to(np.asarray(b1, np.float32), (128, HID)).copy(),
                "w2b": np.broadcast_to(
                    np.asarray(W2, np.float32)[:, 0], (128, HID)
                ).copy(),
                "b2b": np.full((128, 1), np.asarray(b2, np.float32)[0], np.float32),
                "bdiag": np.kron(np.eye(8, dtype=np.float32), np.ones((16, 16), np.float32)),
            }
        )

    consts = dict(n_chunks=n_chunks, B_cap=B_cap, NB=NB, NPP=NPP, NSLOT=NSLOT, VN=VN, SUB=SUB, C=C)
    meta = dict(virt=virt, nc_of=nc_of, NSLOT=NSLOT, NPP=NPP)
    return in_maps, consts, meta


def _build(consts, repeat=1, skip=()):
    import concourse.bacc as bacc
    import concourse.tile as tile
    import concourse.mybir as mybir

    F32 = mybir.dt.float32
    BF16 = mybir.dt.bfloat16
    I16 = mybir.dt.int16
    AOT = mybir.AluOpType
    ACTF = mybir.ActivationFunctionType

    n_chunks = consts["n_chunks"]
    B_cap = consts["B_cap"]
    NB = consts["NB"]
    NPP = consts["NPP"]
    NSLOT = consts["NSLOT"]
    VN = consts["VN"]
    SUB = consts["SUB"]
    C = consts["C"]

    nc = bacc.Bacc("TRN2", target_bir_lowering=False, debug=False, num_devices=N_CORES)

    def inp(name, shape, dt=F32):
        return nc.dram_tensor(name, shape, dt, kind="ExternalInput").ap()

    idx16 = inp("idx16", [n_chunks, 128, C // 16], I16)
    maskb = inp("maskb", [n_chunks, 128, 2 * C], BF16)
    maskf = inp("maskf", [n_chunks, 128, C])
    bidx16 = inp("bidx16", [n_chunks, 128, B_cap // 16], I16)
    pbl_lo = inp("pbl_lo", [128, NPP])
    pbl_hi = inp("pbl_hi", [128, NPP])
    degm1_full = inp("degm1_full", [1024, NPP])
    x_own = inp("x_own", [128, 2 * NPP])
    x_virt = inp("x_virt", [VN, 2])
    w1b0 = inp("w1b0", [128, HID])
    w1b1 = inp("w1b1", [128, HID])
    b1b = inp("b1b", [128, HID])
    w2b = inp("w2b", [128, HID])
    b2b = inp("b2b", [128, 1])
    bdiag = inp("bdiag", [128, 128])

    out_ext = nc.dram_tensor("out", [128, NPP], F32, kind="ExternalOutput").ap()

    with tile.TileContext(nc) as tc:
        with (
            tc.tile_pool(name="big", bufs=1) as big_pool,
            tc.tile_pool(name="gp", bufs=2) as gpool,
            tc.tile_pool(name="mask", bufs=1) as mask_pool,
            tc.tile_pool(name="idx", bufs=3) as idx_pool,
            tc.tile_pool(name="qt", bufs=1) as qt_pool,
            tc.tile_pool(name="qtb", bufs=1) as qtb_pool,
            tc.tile_pool(name="qb", bufs=1) as qb_pool,
            tc.tile_pool(name="node", bufs=1) as node_pool,
            tc.tile_pool(name="psum", bufs=4, space="PSUM") as psum_pool,
            tc.tile_pool(name="dram", bufs=1, space="DRAM") as dram_pool,
        ):
            # ---------- persistent node-layout tiles ----------
            for _rep in range(repeat):
                t_lo = node_pool.tile([128, NPP], F32, tag="t_lo")
                t_hi = node_pool.tile([128, NPP], F32, tag="t_hi")
                t_dinv = node_pool.tile([128, NPP], F32, tag="t_dinv")
                t_z0 = node_pool.tile([128, NPP], F32, tag="t_z0")
                t_z1 = node_pool.tile([128, NPP], F32, tag="t_z1")
                t_g = node_pool.tile([128, NPP], F32, tag="t_g")
                t_xo = node_pool.tile([128, 2 * NPP], F32, tag="t_xo")
                t_w = node_pool.tile([128, 4 * HID + 1], F32, tag="t_w")
                t_bd = node_pool.tile([128, 128], F32, tag="t_bd")
                t_bdb = node_pool.tile([128, 128], BF16, tag="t_bdb")
                t_carry = node_pool.tile([128, 2], F32, tag="t_carry")
                t_zero = node_pool.tile([128, 1], F32, tag="t_zero")
                nc.vector.memset(t_zero[:], 0.0)
                t_out = node_pool.tile([128, NPP], F32, tag="t_out")

                nc.sync.dma_start(out=t_lo[:], in_=pbl_lo[:])
                nc.sync.dma_start(out=t_hi[:], in_=pbl_hi[:])
                nc.sync.dma_start(out=t_xo[:], in_=x_own[:])
                nc.sync.dma_start(out=t_w[:, 0:HID], in_=w1b0[:])
                nc.sync.dma_start(out=t_w[:, HID : 2 * HID], in_=w1b1[:])
                nc.sync.dma_start(out=t_w[:, 2 * HID : 3 * HID], in_=b1b[:])
                nc.sync.dma_start(out=t_w[:, 3 * HID : 4 * HID], in_=w2b[:])
                nc.sync.dma_start(out=t_w[:, 4 * HID : 4 * HID + 1], in_=b2b[:])
                nc.sync.dma_start(out=t_bd[:], in_=bdiag[:])
                nc.vector.tensor_copy(out=t_bdb[:], in_=t_bd[:])

                # deg = hi - lo + 1 ; dinv = rsqrt(deg)
                nc.vector.tensor_tensor(out=t_dinv[:], in0=t_hi[:], in1=t_lo[:], op=AOT.subtract)
                nc.scalar.activation(t_dinv[:], t_dinv[:], ACTF.Sqrt, bias=1.0)
                nc.vector.reciprocal(out=t_dinv[:], in_=t_dinv[:])

                # ---------- full dinv computed locally (no collective) ----------
                # host ships (hi - lo) for ALL cores' nodes in the allgather
                # layout; each NC computes rsqrt(deg) itself
                NV = VN // 128
                t_dfull = node_pool.tile([128, NV], F32, tag="t_dfull")
                nc.sync.dma_start(
                    out=t_dfull[:],
                    in_=degm1_full[:].rearrange("(p a) f -> p (a f)", p=128))
                nc.scalar.activation(t_dfull[:], t_dfull[:], ACTF.Sqrt, bias=1.0)
                nc.vector.reciprocal(out=t_dfull[:], in_=t_dfull[:])

                # ---------- y_full = dinv_full * x_virt -> bf16 (in DRAM) ----------
                d_ybf = dram_pool.tile([VN, 2], BF16, tag="d_ybf")
                t_scr = qtb_pool.tile([128, C + 4], F32, tag="t_qtb")
                xt = t_scr[:, : 2 * NV]
                dt_ = t_dfull[:]
                nc.sync.dma_start(out=xt, in_=x_virt[:].rearrange("(p f) two -> p (f two)", p=128))
                xt3 = xt.rearrange("p (f two) -> p f two", two=2)
                nc.vector.tensor_tensor(out=xt3[:, :, 0], in0=xt3[:, :, 0], in1=dt_, op=AOT.mult)
                nc.vector.tensor_tensor(out=xt3[:, :, 1], in0=xt3[:, :, 1], in1=dt_, op=AOT.mult)
                t_ybf = mask_pool.tile([128, 2 * C], BF16, tag="t_mb")
                nc.vector.tensor_copy(out=t_ybf[:, : 2 * NV], in_=xt)
                nc.sync.dma_start(
                    out=d_ybf[:].rearrange("(p f) two -> p (f two)", p=128),
                    in_=t_ybf[:, : 2 * NV])

                # ---------- helper: one aggregation pass ----------
                def agg_pass(tables, mode, qb_tiles):
                    """mode 'b': bf16-pair gather (d=2 bf16, 2 features);
                    mode 'c': f32 scalar gather (d=1)."""
                    d = 2 if mode == "b" else 1
                    nc.vector.memset(t_carry[:, :d], 0.0)
                    for qb in qb_tiles:
                        nc.vector.memset(qb[:], 0.0)

                    def emit_bgather(kk, qt_prev):
                        # gather scan-table values at node boundaries,
                        # straight from SBUF (no DRAM round trip); slots
                        # into the next stream gather's shadow on GPSIMD
                        t_bidx = idx_pool.tile([128, B_cap // 16], I16, tag="t_bidx")
                        nc.sync.dma_start(out=t_bidx[:], in_=bidx16[kk])
                        for f in range(d):
                            nc.gpsimd.ap_gather(
                                qb_tiles[f][:, kk * B_cap : (kk + 1) * B_cap],
                                qt_prev[f][:],
                                t_bidx[:],
                                channels=128,
                                num_elems=C + 4,
                                d=1,
                                num_idxs=B_cap,
                            )

                    prev_qt = None
                    for k in range(n_chunks):
                        t_idx = idx_pool.tile([128, C // 16], I16, tag="t_idx")
                        nc.sync.dma_start(out=t_idx[:], in_=idx16[k])
                        if mode == "b":
                            t_mask = mask_pool.tile([128, 2 * C], BF16, tag="t_mb")
                            gout = gpool.tile([128, 2 * C], BF16, tag="gB")
                            if "maskdma" not in skip:
                                nc.sync.dma_start(out=t_mask[:], in_=maskb[k])
                            else:
                                nc.vector.memset(t_mask[:], 0.0)
                            nc.gpsimd.ap_gather(
                                gout[:],
                                tables[:],
                                t_idx[:],
                                channels=128,
                                num_elems=SUB,
                                d=2,
                                num_idxs=C,
                            )
                        else:
                            t_mask = mask_pool.tile([128, C], F32, tag="t_mc")
                            gout = gpool.tile([128, C], F32, tag="gC")
                            if "maskdma" not in skip:
                                nc.sync.dma_start(out=t_mask[:], in_=maskf[k])
                            else:
                                nc.vector.memset(t_mask[:], 0.0)
                            nc.gpsimd.ap_gather(
                                gout[:],
                                tables[:],
                                t_idx[:],
                                channels=128,
                                num_elems=SUB,
                                d=1,
                                num_idxs=C,
                            )
                        if prev_qt is not None:
                            emit_bgather(k - 1, prev_qt)
                        nc.vector.tensor_tensor(out=gout[:], in0=gout[:], in1=t_mask[:], op=AOT.mult)
                        # 16->1 reduce via block-diag matmul; deinterleave
                        # bf16 pairs into per-feature f32 scan inputs
                        qt_tiles = []
                        rs_tiles = []
                        for f in range(d):
                            t_qt = qt_pool.tile([128, C + 4], F32, tag=f"t_qt{f}")
                            t_rs = qt_pool.tile([128, C], F32, tag=f"t_rs{f}")
                            qt_tiles.append(t_qt)
                            rs_tiles.append(t_rs)
                            nc.vector.memset(t_qt[:, C + 1 :], 0.0)
                            nc.vector.tensor_copy(out=t_qt[:, 0:1], in_=t_carry[:, f : f + 1])
                        W = d * C
                        nblk = (W + 511) // 512
                        for n in range(nblk):
                            L = min(512, W - n * 512)
                            ps = psum_pool.tile([128, 512], F32)
                            nc.tensor.matmul(
                                out=ps[:, :L],
                                lhsT=t_bdb[:] if mode == "b" else t_bd[:],
                                rhs=gout[:, n * 512 : n * 512 + L],
                                start=True,
                                stop=True,
                            )
                            if mode == "b":
                                ps2 = ps[:, :L].rearrange("p (s two) -> p two s", two=2)
                                for f in range(2):
                                    nc.scalar.activation(
                                        rs_tiles[f][:, n * 256 : n * 256 + L // 2],
                                        ps2[:, f, :],
                                        ACTF.Identity,
                                    )
                            else:
                                nc.scalar.activation(
                                    rs_tiles[0][:, n * 512 : n * 512 + L], ps[:, :L], ACTF.Identity
                                )
                        for f in range(d):
                            t_qt = qt_tiles[f]
                            nc.vector.tensor_tensor_scan(
                                t_qt[:, 1 : C + 1],
                                rs_tiles[f][:],
                                t_zero[:, 0:1].to_broadcast([128, C]),
                                t_qt[:, 0:1],
                                AOT.add,
                                AOT.add,
                            )
                            nc.vector.tensor_copy(out=t_carry[:, f : f + 1], in_=t_qt[:, C : C + 1])
                        prev_qt = qt_tiles
                    emit_bgather(n_chunks - 1, prev_qt)

                # ---------- pass B ----------
                TB = big_pool.tile([128, 2 * SUB], BF16, tag="TB")
                y16 = d_ybf[:].rearrange("(s e) two -> s (e two)", s=16)
                # one DMA replicates y to all 8 groups via 0-stride source
                nc.sync.dma_start(
                    out=TB[:],
                    in_=y16.unsqueeze(0).broadcast_to((8, 16, 2 * SUB)))
                qb0 = qb_pool.tile([128, NSLOT + 4], F32, tag="qb0")
                qb1 = qb_pool.tile([128, NSLOT + 4], F32, tag="qb1")
                agg_pass(TB, "b", [qb0, qb1])

                # ---------- QB -> D (node layout) ----------
                def qb_to_d(qb, t_dst):
                    """t_dst[16c+p, m] = qb[c, p*NPP+m+1] - qb[c, p*NPP+m]."""
                    src8 = qb[:].rearrange("(a b) f -> a b f", b=16)[:, 0, :]
                    t_l = qtb_pool.tile([128, C + 4], F32, tag="t_qtb")
                    nc.sync.dma_start(
                        out=t_l[:, :NPP],
                        in_=src8[:, :NSLOT].rearrange("a (b f) -> a b f", b=16))
                    nc.sync.dma_start(
                        out=t_l[:, NPP : 2 * NPP],
                        in_=src8[:, 1 : NSLOT + 1].rearrange("a (b f) -> a b f", b=16))
                    nc.vector.tensor_tensor(out=t_dst[:], in0=t_l[:, NPP : 2 * NPP], in1=t_l[:, :NPP], op=AOT.subtract)

                qb_to_d(qb0, t_z0)
                qb_to_d(qb1, t_z1)

                # ---------- z = dinv*(D + dinv*x_own) ----------
                xo3 = t_xo[:].rearrange("p (f two) -> p two f", two=2)
                for f, tz in ((0, t_z0), (1, t_z1)):
                    t_tmp = t_out
                    nc.vector.tensor_tensor(out=t_tmp[:], in0=xo3[:, f, :], in1=t_dinv[:], op=AOT.mult)
                    nc.vector.tensor_tensor(out=tz[:], in0=tz[:], in1=t_tmp[:], op=AOT.add)
                    nc.vector.tensor_tensor(out=tz[:], in0=tz[:], in1=t_dinv[:], op=AOT.mult)

                # ---------- h1 = relu(z @ W1 + b1); g = h1 @ W2 ----------
                # two node-halves so the scratch fits one [128, SUB] slot
                assert NPP % 2 == 0
                HL = NPP // 2
                mm = big_pool.tile([128, SUB], F32, tag="TB")
                for o in (0, HL):
                    h = mm[:, : HID * HL].rearrange("p (k f) -> p k f", k=HID)
                    tmp = mm[:, HID * HL : 2 * HID * HL].rearrange("p (k f) -> p k f", k=HID)
                    z0b = t_z0[:, o : o + HL].unsqueeze(1).broadcast_to((128, HID, HL))
                    z1b = t_z1[:, o : o + HL].unsqueeze(1).broadcast_to((128, HID, HL))
                    w0b = t_w[:, 0:HID].unsqueeze(2).broadcast_to((128, HID, HL))
                    w1b = t_w[:, HID : 2 * HID].unsqueeze(2).broadcast_to((128, HID, HL))
                    bb = t_w[:, 2 * HID : 3 * HID].unsqueeze(2).broadcast_to((128, HID, HL))
                    w2bb = t_w[:, 3 * HID : 4 * HID].unsqueeze(2).broadcast_to((128, HID, HL))
                    nc.vector.tensor_tensor(out=h, in0=z0b, in1=w0b, op=AOT.mult)
                    nc.vector.tensor_tensor(out=tmp, in0=z1b, in1=w1b, op=AOT.mult)
                    nc.vector.tensor_tensor(out=h, in0=h, in1=tmp, op=AOT.add)
                    nc.vector.tensor_tensor(out=h, in0=h, in1=bb, op=AOT.add)
                    nc.vector.tensor_scalar_max(h, h, 0.0)
                    nc.vector.tensor_tensor(out=h, in0=h, in1=w2bb, op=AOT.mult)
                    nc.vector.tensor_reduce(
                        out=t_g[:, o : o + HL],
                        in_=mm[:, : HID * HL].rearrange("p (k f) -> p f k", k=HID),
                        axis=mybir.AxisListType.X,
                        op=AOT.add,
                    )

                # ---------- allgather g; gy_full = dinv_full * g_full ----------
                g_slice = dram_pool.tile([128, NPP], F32, tag="g_slice")
                g_full = dram_pool.tile([1024, NPP], F32, tag="g_full")
                nc.sync.dma_start(out=g_slice[:], in_=t_g[:])
                if "coll" in skip:
                    for _j in range(N_CORES):
                        nc.sync.dma_start(
                            out=g_full[_j * 128 : (_j + 1) * 128, :], in_=g_slice[:])
                else:
                    nc.gpsimd.collective_compute(
                        "AllGather",
                        AOT.bypass,
                        replica_groups=[list(range(N_CORES))],
                        ins=[g_slice[:].opt()],
                        outs=[g_full[:].opt()],
                    )
                d_gy = dram_pool.tile([VN], F32, tag="d_gy")
                t_scr2 = qtb_pool.tile([128, C + 4], F32, tag="t_qtb")
                gt = t_scr2[:, :NV]
                nc.sync.dma_start(out=gt, in_=g_full[:].rearrange("(p a) f -> p (a f)", p=128))
                nc.vector.tensor_tensor(out=gt, in0=gt, in1=t_dfull[:], op=AOT.mult)
                nc.sync.dma_start(out=d_gy[:].rearrange("(p f) -> p f", p=128), in_=gt)

                # ---------- pass C ----------
                TC = big_pool.tile([128, SUB], F32, tag="TB")
                gy16 = d_gy[:].rearrange("(s e) -> s e", s=16)
                nc.sync.dma_start(
                    out=TC[:],
                    in_=gy16.unsqueeze(0).broadcast_to((8, 16, SUB)))
                qbc = qb_pool.tile([128, NSLOT + 4], F32, tag="qb0")
                agg_pass(TC, "c", [qbc])
                t_dc = t_z0
                qb_to_d(qbc, t_dc)

                # ---------- out = dinv*(D' + dinv*g) + b2 ----------
                nc.vector.tensor_tensor(out=t_out[:], in0=t_g[:], in1=t_dinv[:], op=AOT.mult)
                nc.vector.tensor_tensor(out=t_out[:], in0=t_out[:], in1=t_dc[:], op=AOT.add)
                nc.vector.tensor_tensor(out=t_out[:], in0=t_out[:], in1=t_dinv[:], op=AOT.mult)
                nc.vector.tensor_tensor(
                    out=t_out[:], in0=t_out[:], in1=t_w[:, 4 * HID : 4 * HID + 1].to_broadcast([128, NPP]), op=AOT.add
                )
                nc.sync.dma_start(out=out_ext[:], in_=t_out[:])

    nc.compile()
    return nc


def kernel(x, edge_index, W1, b1, W2, b2):
    from concourse.bass_utils import run_bass_kernel_spmd

    in_maps, consts, meta = _prep(x, edge_index, W1, b1, W2, b2)
    key = tuple(sorted(consts.items()))
    if key not in _cache:
        _cache[key] = _build(consts)
    nc = _cache[key]
    res = run_bass_kernel_spmd(nc, in_maps, list(range(N_CORES)))
    virt = meta["virt"]
    NSLOT = meta["NSLOT"]
    NPP = meta["NPP"]
    out_full = np.zeros(64 * NSLOT, dtype=np.float32)
    for i in range(N_CORES):
        out_full[i * 8 * NSLOT : (i + 1) * 8 * NSLOT] = res.results[i]["out"].reshape(-1)
    return out_full[virt].astype(np.float32)



# revision 50
# speedup vs baseline: 1.1801x; 1.1801x over previous
"""GCN (2-layer, PyG GCNConv-style) on 8 Trainium2 NeuronCores.

Strategy (1D destination partition, per sharding hint):
  - Nodes are relabeled into a "virtual" order: 8 NCs x 8 Q7-cores x NSLOT
    slots. Each (NC, core) owns ~1563 original nodes.
  - Edges are grouped by destination core ("edge lists grouped by
    destination-node partition") and sorted by destination within the core.
  - GCNConv is linear before the nonlinearity, so aggregation happens in the
    2-dim input space (layer 1: aggregate dinv*x, then @W1) and in the 1-dim
    output space (layer 2: aggregate dinv*(h1@W2)).
  - Per-edge gather of source values runs on GPSIMD ap_gather with sixteen
    per-partition sub-tables; a shipped 0/1 mask + one block-diagonal PE
    matmul select the correct sub-table and reduce 16 partitions -> 1 row.
    Pass B packs the 2-dim y values as bf16 pairs (one 4-byte gather unit
    per edge instead of two), halving the ucode's per-index inner loop.
  - Segment sums use chunked prefix scans (DVE) over per-core streams plus
    boundary gathers of the scan table; destination degrees come from
    boundary differences. Chunk size C is fitted to the largest per-core
    stream (minimal padding); gather outputs are double-buffered and the
    boundary-gather reloads alternate between two SBUF slots so DMA
    overlaps GPSIMD.
  - dinv and g are exchanged across the 8 cores with AllGather collectives.
Host code does only data movement: permutations, grouping, padding, index
tables, and broadcast of the tiny weights.
"""

import math

import numpy as np

N_CORES = 8
N = 100_000
IN_DIM = 2
HID = 64
C_TARGET = 3584

_cache = {}


def _ceil16(x):
    return ((x + 15) // 16) * 16


def _prep(x, edge_index, W1, b1, W2, b2):
    row = np.asarray(edge_index[0], dtype=np.int64)
    col = np.asarray(edge_index[1], dtype=np.int64)
    E = row.shape[0]

    # ---- node -> (nc, core, j) assignment ----
    per_nc = (N + N_CORES - 1) // N_CORES  # 12500
    nd_core = np.full(8, per_nc // 8, dtype=np.int64)
    nd_core[: per_nc % 8] += 1  # [1563]*4 + [1562]*4
    cum_nd = np.concatenate([[0], np.cumsum(nd_core)])  # [9]

    v = np.arange(N, dtype=np.int64)
    nc_of = v // per_nc
    l_of = v % per_nc
    core_of = np.searchsorted(cum_nd, l_of, side="right") - 1
    j_of = l_of - cum_nd[core_of]
    cg_of = nc_of * 8 + core_of  # global core id [0,64)

    # ---- edge stream: group by dest core, sort by dest j ----
    e_cg = cg_of[col]
    e_j = j_of[col]
    order = np.lexsort((e_j, e_cg))
    s_cg = e_cg[order]
    s_j = e_j[order]
    s_src = row[order]

    S_real = np.bincount(s_cg, minlength=64)
    cg_start = np.concatenate([[0], np.cumsum(S_real)])
    # tight chunking: same chunk count as C_TARGET would give, but C
    # shrunk to just cover the largest per-core stream (less padding)
    maxS = int(S_real.max())
    n_chunks = int(math.ceil(maxS / C_TARGET))
    C = _ceil16(int(math.ceil(maxS / n_chunks)))
    S_pad = n_chunks * C

    # ---- boundaries per core ----
    # counts per (cg, j); nd = nd_core[c]
    bounds = []  # per cg: array length nd+1
    for cg in range(64):
        c = cg % 8
        nd = int(nd_core[c])
        jj = s_j[cg_start[cg] : cg_start[cg + 1]]
        cnt = np.bincount(jj, minlength=nd)
        bounds.append(np.concatenate([[0], np.cumsum(cnt)]))

    # chunk assignment + B_cap
    maxb = 0
    for cg in range(64):
        b = bounds[cg]
        kb = np.minimum(b // C, n_chunks - 1)
        maxb = max(maxb, int(np.bincount(kb, minlength=n_chunks).max()))
    B_cap = _ceil16(maxb + 2)
    NB = n_chunks * B_cap
    NPP = (NB + 15) // 16
    NSLOT = 16 * NPP
    VN = 64 * NSLOT
    SUB = VN // 16
    assert SUB * 2 <= 32768, (SUB, NB)

    # ---- padded boundary lists (PBL), positions, virtual ids ----
    PBL = np.zeros((64, NB), dtype=np.int64)
    pos_of = np.zeros((64,), dtype=object)
    for cg in range(64):
        b = bounds[cg]
        kb = np.minimum(b // C, n_chunks - 1)
        cnts = np.bincount(kb, minlength=n_chunks)
        lists = []
        last_val = 0
        start = 0
        for k in range(n_chunks):
            ck = int(cnts[k])
            vals = b[start : start + ck]
            start += ck
            if ck > 0:
                last_val = int(vals[-1])
                padv = last_val
            else:
                padv = max(k * C, last_val)
            lst = np.concatenate([vals, np.full(B_cap - ck, padv, dtype=np.int64)])
            lists.append(lst)
        PBL[cg] = np.concatenate(lists)
        # entry position of b[j] in PBL: P[j] = j + padcum[kb[j]]
        pads = B_cap - cnts
        padcum = np.concatenate([[0], np.cumsum(pads)])[:-1]
        P = np.arange(len(b)) + padcum[kb]
        pos = P[1:] - 1  # pos_j for j = 0..nd-1
        assert pos.max() <= NB - 2, (cg, pos.max(), NB)
        pos_of[cg] = pos

    # virtual id per original node
    virt = np.zeros(N, dtype=np.int64)
    for cg in range(64):
        c = cg % 8
        nd = int(nd_core[c])
        sel = cg_of == cg
        virt[sel] = cg * NSLOT + pos_of[cg][j_of[sel]]

    # ---- per-edge source virtual ids, padded streams ----
    su = virt[s_src]
    su_stream = np.zeros((64, S_pad), dtype=np.int64)
    for cg in range(64):
        n = int(S_real[cg])
        su_stream[cg, :n] = su[cg_start[cg] : cg_start[cg + 1]]

    # ---- shipped arrays per NC ----
    x = np.asarray(x, dtype=np.float32)
    x_virt = np.zeros((VN, 2), dtype=np.float32)
    x_virt[virt] = x

    qv = (su_stream // SUB).astype(np.int64)  # [64, S_pad] in [0,16)
    idxv = (su_stream % SUB).astype(np.int16)

    import ml_dtypes

    # (hi - lo) per virtual slot for ALL 64 cores, in the [1024, NPP]
    # layout an AllGather of per-core [128, NPP] dinv shards would produce
    degm1_full = np.zeros((1024, NPP), dtype=np.float32)
    for i2 in range(N_CORES):
        for c2 in range(8):
            cg2 = i2 * 8 + c2
            pbl_e = np.concatenate([PBL[cg2], PBL[cg2][-1:]])
            dm1 = (pbl_e[1 : NSLOT + 1] - pbl_e[:NSLOT]).astype(np.float32)
            degm1_full[i2 * 128 + 16 * c2 : i2 * 128 + 16 * c2 + 16] = (
                dm1.reshape(16, NPP)
            )

    in_maps = []
    for i in range(N_CORES):
        idx16 = np.zeros((n_chunks, 128, C // 16), dtype=np.int16)
        maskf = np.zeros((n_chunks, 128, C), dtype=np.float32)
        bidx16 = np.zeros((n_chunks, 128, B_cap // 16), dtype=np.int16)
        lo = np.zeros((128, NPP), dtype=np.float32)
        hi = np.zeros((128, NPP), dtype=np.float32)
        x_own = np.zeros((128, 2 * NPP), dtype=np.float32)
        for c in range(8):
            cg = i * 8 + c
            for k in range(n_chunks):
                chunk_idx = idxv[cg, k * C : (k + 1) * C].reshape(C // 16, 16)
                idx16[k, 16 * c : 16 * c + 16, :] = chunk_idx.T
                qk = qv[cg, k * C : (k + 1) * C]
                # mask[16c+p, s] = (q[s] == p), 0 for dummy slots
                s_valid = (np.arange(k * C, (k + 1) * C) < S_real[cg]).astype(
                    np.float32
                )
                m = (qk[None, :] == np.arange(16)[:, None]).astype(np.float32)
                maskf[k, 16 * c : 16 * c + 16, :] = m * s_valid[None, :]
                pb = PBL[cg, k * B_cap : (k + 1) * B_cap] - k * C
                assert pb.min() >= 0 and pb.max() <= C, (cg, k)
                bidx16[k, 16 * c : 16 * c + 16, :] = (
                    pb.astype(np.int16).reshape(B_cap // 16, 16).T
                )
            pbl_ext = np.concatenate([PBL[cg], PBL[cg][-1:]])
            lo_full = pbl_ext[:NSLOT].astype(np.float32)
            hi_full = pbl_ext[1 : NSLOT + 1].astype(np.float32)
            lo[16 * c : 16 * c + 16] = lo_full.reshape(16, NPP)
            hi[16 * c : 16 * c + 16] = hi_full.reshape(16, NPP)
            x_own[16 * c : 16 * c + 16] = x_virt[
                cg * NSLOT : (cg + 1) * NSLOT
            ].reshape(16, 2 * NPP)
        # pass-B mask in bf16 with each slot duplicated for the packed
        # (y0, y1) bf16 pair layout of the gathered stream
        maskb = np.repeat(maskf, 2, axis=-1).astype(ml_dtypes.bfloat16)
        in_maps.append(
            {
                "idx16": idx16,
                "maskb": maskb,
                "maskf": maskf,
                "bidx16": bidx16,
                "pbl_lo": lo,
                "pbl_hi": hi,
                "x_own": x_own,
                "x_virt": x_virt,
                "degm1_full": degm1_full,
                "w1b0": np.broadcast_to(
                    np.asarray(W1, np.float32)[0], (128, HID)
                ).copy(),
                "w1b1": np.broadcast_to(
                    np.asarray(W1, np.float32)[1], (128, HID)
                ).copy(),
                "b1b": np.broadcast_to(np.asarray(b1, np.float32), (128, HID)).copy(),
                "w2b": np.broadcast_to(
                    np.asarray(W2, np.float32)[:, 0], (128, HID)
                ).copy(),
                "b2b": np.full((128, 1), np.asarray(b2, np.float32)[0], np.float32),
                "bdiag": np.kron(np.eye(8, dtype=np.float32), np.ones((16, 16), np.float32)),
            }
        )

    consts = dict(n_chunks=n_chunks, B_cap=B_cap, NB=NB, NPP=NPP, NSLOT=NSLOT, VN=VN, SUB=SUB, C=C)
    meta = dict(virt=virt, nc_of=nc_of, NSLOT=NSLOT, NPP=NPP)
    return in_maps, consts, meta


def _build(consts, repeat=1, skip=()):
    import concourse.bacc as bacc
    import concourse.tile as tile
    import concourse.mybir as mybir

    F32 = mybir.dt.float32
    BF16 = mybir.dt.bfloat16
    I16 = mybir.dt.int16
    AOT = mybir.AluOpType
    ACTF = mybir.ActivationFunctionType

    n_chunks = consts["n_chunks"]
    B_cap = consts["B_cap"]
    NB = consts["NB"]
    NPP = consts["NPP"]
    NSLOT = consts["NSLOT"]
    VN = consts["VN"]
    SUB = consts["SUB"]
    C = consts["C"]

    nc = bacc.Bacc("TRN2", target_bir_lowering=False, debug=False, num_devices=N_CORES)

    def inp(name, shape, dt=F32):
        return nc.dram_tensor(name, shape, dt, kind="ExternalInput").ap()

    idx16 = inp("idx16", [n_chunks, 128, C // 16], I16)
    maskb = inp("maskb", [n_chunks, 128, 2 * C], BF16)
    maskf = inp("maskf", [n_chunks, 128, C])
    bidx16 = inp("bidx16", [n_chunks, 128, B_cap // 16], I16)
    pbl_lo = inp("pbl_lo", [128, NPP])
    pbl_hi = inp("pbl_hi", [128, NPP])
    degm1_full = inp("degm1_full", [1024, NPP])
    x_own = inp("x_own", [128, 2 * NPP])
    x_virt = inp("x_virt", [VN, 2])
    w1b0 = inp("w1b0", [128, HID])
    w1b1 = inp("w1b1", [128, HID])
    b1b = inp("b1b", [128, HID])
    w2b = inp("w2b", [128, HID])
    b2b = inp("b2b", [128, 1])
    bdiag = inp("bdiag", [128, 128])

    out_ext = nc.dram_tensor("out", [128, NPP], F32, kind="ExternalOutput").ap()

    with tile.TileContext(nc) as tc:
        with (
            tc.tile_pool(name="big", bufs=1) as big_pool,
            tc.tile_pool(name="gp", bufs=2) as gpool,
            tc.tile_pool(name="mask", bufs=1) as mask_pool,
            tc.tile_pool(name="idx", bufs=2) as idx_pool,
            tc.tile_pool(name="qt", bufs=1) as qt_pool,
            tc.tile_pool(name="qtb", bufs=1) as qtb_pool,
            tc.tile_pool(name="qb", bufs=1) as qb_pool,
            tc.tile_pool(name="node", bufs=1) as node_pool,
            tc.tile_pool(name="psum", bufs=4, space="PSUM") as psum_pool,
            tc.tile_pool(name="dram", bufs=1, space="DRAM") as dram_pool,
        ):
            # ---------- persistent node-layout tiles ----------
            for _rep in range(repeat):
                t_lo = node_pool.tile([128, NPP], F32, tag="t_lo")
                t_hi = node_pool.tile([128, NPP], F32, tag="t_hi")
                t_dinv = node_pool.tile([128, NPP], F32, tag="t_dinv")
                t_z0 = node_pool.tile([128, NPP], F32, tag="t_z0")
                t_z1 = node_pool.tile([128, NPP], F32, tag="t_z1")
                t_g = node_pool.tile([128, NPP], F32, tag="t_g")
                t_xo = node_pool.tile([128, 2 * NPP], F32, tag="t_xo")
                t_w = node_pool.tile([128, 4 * HID + 1], F32, tag="t_w")
                t_bd = node_pool.tile([128, 128], F32, tag="t_bd")
                t_bdb = node_pool.tile([128, 128], BF16, tag="t_bdb")
                t_carry = node_pool.tile([128, 2], F32, tag="t_carry")
                t_zero = node_pool.tile([128, 1], F32, tag="t_zero")
                nc.vector.memset(t_zero[:], 0.0)
                t_out = node_pool.tile([128, NPP], F32, tag="t_out")

                nc.sync.dma_start(out=t_lo[:], in_=pbl_lo[:])
                nc.sync.dma_start(out=t_hi[:], in_=pbl_hi[:])
                nc.sync.dma_start(out=t_xo[:], in_=x_own[:])
                nc.sync.dma_start(out=t_w[:, 0:HID], in_=w1b0[:])
                nc.sync.dma_start(out=t_w[:, HID : 2 * HID], in_=w1b1[:])
                nc.sync.dma_start(out=t_w[:, 2 * HID : 3 * HID], in_=b1b[:])
                nc.sync.dma_start(out=t_w[:, 3 * HID : 4 * HID], in_=w2b[:])
                nc.sync.dma_start(out=t_w[:, 4 * HID : 4 * HID + 1], in_=b2b[:])
                nc.sync.dma_start(out=t_bd[:], in_=bdiag[:])
                nc.vector.tensor_copy(out=t_bdb[:], in_=t_bd[:])

                # deg = hi - lo + 1 ; dinv = rsqrt(deg)
                nc.vector.tensor_tensor(out=t_dinv[:], in0=t_hi[:], in1=t_lo[:], op=AOT.subtract)
                nc.scalar.activation(t_dinv[:], t_dinv[:], ACTF.Sqrt, bias=1.0)
                nc.vector.reciprocal(out=t_dinv[:], in_=t_dinv[:])

                # ---------- full dinv computed locally (no collective) ----------
                # host ships (hi - lo) for ALL cores' nodes in the allgather
                # layout; each NC computes rsqrt(deg) itself
                NV = VN // 128
                t_dfull = node_pool.tile([128, NV], F32, tag="t_dfull")
                nc.sync.dma_start(
                    out=t_dfull[:],
                    in_=degm1_full[:].rearrange("(p a) f -> p (a f)", p=128))
                nc.scalar.activation(t_dfull[:], t_dfull[:], ACTF.Sqrt, bias=1.0)
                nc.vector.reciprocal(out=t_dfull[:], in_=t_dfull[:])

                # ---------- y_full = dinv_full * x_virt -> bf16 (in DRAM) ----------
                d_ybf = dram_pool.tile([VN, 2], BF16, tag="d_ybf")
                t_scr = qtb_pool.tile([128, C + 4], F32, tag="t_qtb")
                xt = t_scr[:, : 2 * NV]
                dt_ = t_dfull[:]
                nc.sync.dma_start(out=xt, in_=x_virt[:].rearrange("(p f) two -> p (f two)", p=128))
                xt3 = xt.rearrange("p (f two) -> p f two", two=2)
                nc.vector.tensor_tensor(out=xt3[:, :, 0], in0=xt3[:, :, 0], in1=dt_, op=AOT.mult)
                nc.vector.tensor_tensor(out=xt3[:, :, 1], in0=xt3[:, :, 1], in1=dt_, op=AOT.mult)
                t_ybf = mask_pool.tile([128, 2 * C], BF16, tag="t_mb")
                nc.vector.tensor_copy(out=t_ybf[:, : 2 * NV], in_=xt)
                nc.sync.dma_start(
                    out=d_ybf[:].rearrange("(p f) two -> p (f two)", p=128),
                    in_=t_ybf[:, : 2 * NV])

                # ---------- helper: one aggregation pass ----------
                def agg_pass(tables, mode, qb_tiles):
                    """mode 'b': bf16-pair gather (d=2 bf16, 2 features);
                    mode 'c': f32 scalar gather (d=1)."""
                    d = 2 if mode == "b" else 1
                    nc.vector.memset(t_carry[:, :d], 0.0)
                    for qb in qb_tiles:
                        nc.vector.memset(qb[:], 0.0)

                    def emit_bgather(kk, qt_prev):
                        # gather scan-table values at node boundaries,
                        # straight from SBUF (no DRAM round trip); slots
                        # into the next stream gather's shadow on GPSIMD
                        t_bidx = idx_pool.tile([128, B_cap // 16], I16, tag="t_bidx")
                        nc.sync.dma_start(out=t_bidx[:], in_=bidx16[kk])
                        for f in range(d):
                            nc.gpsimd.ap_gather(
                                qb_tiles[f][:, kk * B_cap : (kk + 1) * B_cap],
                                qt_prev[f][:],
                                t_bidx[:],
                                channels=128,
                                num_elems=C + 4,
                                d=1,
                                num_idxs=B_cap,
                            )

                    prev_qt = None
                    for k in range(n_chunks):
                        t_idx = idx_pool.tile([128, C // 16], I16, tag="t_idx")
                        nc.sync.dma_start(out=t_idx[:], in_=idx16[k])
                        if mode == "b":
                            t_mask = mask_pool.tile([128, 2 * C], BF16, tag="t_mb")
                            gout = gpool.tile([128, 2 * C], BF16, tag="gB")
                            if "maskdma" not in skip:
                                nc.sync.dma_start(out=t_mask[:], in_=maskb[k])
                            else:
                                nc.vector.memset(t_mask[:], 0.0)
                            nc.gpsimd.ap_gather(
                                gout[:],
                                tables[:],
                                t_idx[:],
                                channels=128,
                                num_elems=SUB,
                                d=2,
                                num_idxs=C,
                            )
                        else:
                            t_mask = mask_pool.tile([128, C], F32, tag="t_mc")
                            gout = gpool.tile([128, C], F32, tag="gC")
                            if "maskdma" not in skip:
                                nc.sync.dma_start(out=t_mask[:], in_=maskf[k])
                            else:
                                nc.vector.memset(t_mask[:], 0.0)
                            nc.gpsimd.ap_gather(
                                gout[:],
                                tables[:],
                                t_idx[:],
                                channels=128,
                                num_elems=SUB,
                                d=1,
                                num_idxs=C,
                            )
                        if prev_qt is not None:
                            emit_bgather(k - 1, prev_qt)
                        nc.vector.tensor_tensor(out=gout[:], in0=gout[:], in1=t_mask[:], op=AOT.mult)
                        # 16->1 reduce via block-diag matmul; deinterleave
                        # bf16 pairs into per-feature f32 scan inputs
                        qt_tiles = []
                        rs_tiles = []
                        for f in range(d):
                            t_qt = qt_pool.tile([128, C + 4], F32, tag=f"t_qt{f}")
                            t_rs = qt_pool.tile([128, C], F32, tag=f"t_rs{f}")
                            qt_tiles.append(t_qt)
                            rs_tiles.append(t_rs)
                            nc.vector.memset(t_qt[:, C + 1 :], 0.0)
                            nc.vector.tensor_copy(out=t_qt[:, 0:1], in_=t_carry[:, f : f + 1])
                        W = d * C
                        nblk = (W + 511) // 512
                        for n in range(nblk):
                            L = min(512, W - n * 512)
                            ps = psum_pool.tile([128, 512], F32)
                            nc.tensor.matmul(
                                out=ps[:, :L],
                                lhsT=t_bdb[:] if mode == "b" else t_bd[:],
                                rhs=gout[:, n * 512 : n * 512 + L],
                                start=True,
                                stop=True,
                            )
                            if mode == "b":
                                ps2 = ps[:, :L].rearrange("p (s two) -> p two s", two=2)
                                for f in range(2):
                                    nc.scalar.activation(
                                        rs_tiles[f][:, n * 256 : n * 256 + L // 2],
                                        ps2[:, f, :],
                                        ACTF.Identity,
                                    )
                            else:
                                nc.scalar.activation(
                                    rs_tiles[0][:, n * 512 : n * 512 + L], ps[:, :L], ACTF.Identity
                                )
                        for f in range(d):
                            t_qt = qt_tiles[f]
                            nc.vector.tensor_tensor_scan(
                                t_qt[:, 1 : C + 1],
                                rs_tiles[f][:],
                                t_zero[:, 0:1].to_broadcast([128, C]),
                                t_qt[:, 0:1],
                                AOT.add,
                                AOT.add,
                            )
                            nc.vector.tensor_copy(out=t_carry[:, f : f + 1], in_=t_qt[:, C : C + 1])
                        prev_qt = qt_tiles
                    emit_bgather(n_chunks - 1, prev_qt)

                # ---------- pass B ----------
                TB = big_pool.tile([128, 2 * SUB], BF16, tag="TB")
                y16 = d_ybf[:].rearrange("(s e) two -> s (e two)", s=16)
                # one DMA replicates y to all 8 groups via 0-stride source
                nc.sync.dma_start(
                    out=TB[:],
                    in_=y16.unsqueeze(0).broadcast_to((8, 16, 2 * SUB)))
                qb0 = qb_pool.tile([128, NSLOT + 4], F32, tag="qb0")
                qb1 = qb_pool.tile([128, NSLOT + 4], F32, tag="qb1")
                agg_pass(TB, "b", [qb0, qb1])

                # ---------- QB -> D (node layout) ----------
                def qb_to_d(qb, t_dst):
                    """t_dst[16c+p, m] = qb[c, p*NPP+m+1] - qb[c, p*NPP+m]."""
                    src8 = qb[:].rearrange("(a b) f -> a b f", b=16)[:, 0, :]
                    t_l = qtb_pool.tile([128, C + 4], F32, tag="t_qtb")
                    nc.sync.dma_start(
                        out=t_l[:, :NPP],
                        in_=src8[:, :NSLOT].rearrange("a (b f) -> a b f", b=16))
                    nc.sync.dma_start(
                        out=t_l[:, NPP : 2 * NPP],
                        in_=src8[:, 1 : NSLOT + 1].rearrange("a (b f) -> a b f", b=16))
                    nc.vector.tensor_tensor(out=t_dst[:], in0=t_l[:, NPP : 2 * NPP], in1=t_l[:, :NPP], op=AOT.subtract)

                qb_to_d(qb0, t_z0)
                qb_to_d(qb1, t_z1)

                # ---------- z = dinv*(D + dinv*x_own) ----------
                xo3 = t_xo[:].rearrange("p (f two) -> p two f", two=2)
                for f, tz in ((0, t_z0), (1, t_z1)):
                    t_tmp = t_out
                    nc.vector.tensor_tensor(out=t_tmp[:], in0=xo3[:, f, :], in1=t_dinv[:], op=AOT.mult)
                    nc.vector.tensor_tensor(out=tz[:], in0=tz[:], in1=t_tmp[:], op=AOT.add)
                    nc.vector.tensor_tensor(out=tz[:], in0=tz[:], in1=t_dinv[:], op=AOT.mult)

                # ---------- h1 = relu(z @ W1 + b1); g = h1 @ W2 ----------
                # two node-halves so the scratch fits one [128, SUB] slot
                assert NPP % 2 == 0
                HL = NPP // 2
                mm = big_pool.tile([128, SUB], F32, tag="TB")
                for o in (0, HL):
                    h = mm[:, : HID * HL].rearrange("p (k f) -> p k f", k=HID)
                    tmp = mm[:, HID * HL : 2 * HID * HL].rearrange("p (k f) -> p k f", k=HID)
                    z0b = t_z0[:, o : o + HL].unsqueeze(1).broadcast_to((128, HID, HL))
                    z1b = t_z1[:, o : o + HL].unsqueeze(1).broadcast_to((128, HID, HL))
                    w0b = t_w[:, 0:HID].unsqueeze(2).broadcast_to((128, HID, HL))
                    w1b = t_w[:, HID : 2 * HID].unsqueeze(2).broadcast_to((128, HID, HL))
                    bb = t_w[:, 2 * HID : 3 * HID].unsqueeze(2).broadcast_to((128, HID, HL))
                    w2bb = t_w[:, 3 * HID : 4 * HID].unsqueeze(2).broadcast_to((128, HID, HL))
                    nc.vector.tensor_tensor(out=h, in0=z0b, in1=w0b, op=AOT.mult)
                    nc.vector.tensor_tensor(out=tmp, in0=z1b, in1=w1b, op=AOT.mult)
                    nc.vector.tensor_tensor(out=h, in0=h, in1=tmp, op=AOT.add)
                    nc.vector.tensor_tensor(out=h, in0=h, in1=bb, op=AOT.add)
                    nc.vector.tensor_scalar_max(h, h, 0.0)
                    nc.vector.tensor_tensor(out=h, in0=h, in1=w2bb, op=AOT.mult)
                    nc.vector.tensor_reduce(
                        out=t_g[:, o : o + HL],
                        in_=mm[:, : HID * HL].rearrange("p (k f) -> p f k", k=HID),
                        axis=mybir.AxisListType.X,
                        op=AOT.add,
                    )

                # ---------- allgather g; gy_full = dinv_full * g_full ----------
                g_slice = dram_pool.tile([128, NPP], F32, tag="g_slice")
                g_full = dram_pool.tile([1024, NPP], F32, tag="g_full")
                nc.sync.dma_start(out=g_slice[:], in_=t_g[:])
                if "coll" in skip:
                    for _j in range(N_CORES):
                        nc.sync.dma_start(
                            out=g_full[_j * 128 : (_j + 1) * 128, :], in_=g_slice[:])
                else:
                    nc.gpsimd.collective_compute(
                        "AllGather",
                        AOT.bypass,
                        replica_groups=[list(range(N_CORES))],
                        ins=[g_slice[:].opt()],
                        outs=[g_full[:].opt()],
                    )
                d_gy = dram_pool.tile([VN], F32, tag="d_gy")
                t_scr2 = qtb_pool.tile([128, C + 4], F32, tag="t_qtb")
                gt = t_scr2[:, :NV]
                nc.sync.dma_start(out=gt, in_=g_full[:].rearrange("(p a) f -> p (a f)", p=128))
                nc.vector.tensor_tensor(out=gt, in0=gt, in1=t_dfull[:], op=AOT.mult)
                nc.sync.dma_start(out=d_gy[:].rearrange("(p f) -> p f", p=128), in_=gt)

                # ---------- pass C ----------
                TC = big_pool.tile([128, SUB], F32, tag="TB")
                gy16 = d_gy[:].rearrange("(s e) -> s e", s=16)
                nc.sync.dma_start(
                    out=TC[:],
                    in_=gy16.unsqueeze(0).broadcast_to((8, 16, SUB)))
                qbc = qb_pool.tile([128, NSLOT + 4], F32, tag="qb0")
                agg_pass(TC, "c", [qbc])
                t_dc = t_z0
                qb_to_d(qbc, t_dc)

                # ---------- out = dinv*(D' + dinv*g) + b2 ----------
                nc.vector.tensor_tensor(out=t_out[:], in0=t_g[:], in1=t_dinv[:], op=AOT.mult)
                nc.vector.tensor_tensor(out=t_out[:], in0=t_out[:], in1=t_dc[:], op=AOT.add)
                nc.vector.tensor_tensor(out=t_out[:], in0=t_out[:], in1=t_dinv[:], op=AOT.mult)
                nc.vector.tensor_tensor(
                    out=t_out[:], in0=t_out[:], in1=t_w[:, 4 * HID : 4 * HID + 1].to_broadcast([128, NPP]), op=AOT.add
                )
                nc.sync.dma_start(out=out_ext[:], in_=t_out[:])

    nc.compile()
    return nc


def kernel(x, edge_index, W1, b1, W2, b2):
    from concourse.bass_utils import run_bass_kernel_spmd

    in_maps, consts, meta = _prep(x, edge_index, W1, b1, W2, b2)
    key = tuple(sorted(consts.items()))
    if key not in _cache:
        _cache[key] = _build(consts)
    nc = _cache[key]
    res = run_bass_kernel_spmd(nc, in_maps, list(range(N_CORES)))
    virt = meta["virt"]
    NSLOT = meta["NSLOT"]
    NPP = meta["NPP"]
    out_full = np.zeros(64 * NSLOT, dtype=np.float32)
    for i in range(N_CORES):
        out_full[i * 8 * NSLOT : (i + 1) * 8 * NSLOT] = res.results[i]["out"].reshape(-1)
    return out_full[virt].astype(np.float32)

